# revision 1
# baseline (speedup 1.0000x reference)
"""Trainium2 Bass kernel for nn_EntropyLoss (256-bin histogram entropy diff).

Data-parallel over 8 NeuronCores: each core histograms 8 of the 64 batch
entries of both tensors (4,194,304 f32 elements per tensor per core, laid
out [128, 32768]); the host sums exact per-core integer counts and computes
the fp32 entropy diff with the reference's own formula (op-for-op on CPU).

Binning is a bit-exact replication of the reference:
  idx = floor((x + 1.0f) * 128.0f), counted only for -1 <= x <= 1
via  u = (x+1)*128  (fp32, dual-op tensor_scalar — same rounding as the
reference's divide-by-width since width is an exact power of two),
  r = int16(u)      (hardware cast, round-half-even),
  m = (u < r), j = r - m   == floor(u) exactly for every fp32 input.

Counting is split across two engines working concurrently:
  - VectorE: bins 0..D-1 by tensor_scalar(is_equal k) with accum_out
    (measured ~9.4us per [128,8192] pass).
  - ScalarE: bins D..255 via cumulative counts C_k = #{j >= k-0.5}, obtained
    from activation(Sign, bias=-(k-0.5)) with accum_out (measured ~5.3us per
    pass; Sign never sees 0 since j is integer).  hist_k = C_k - C_{k+1};
    out-of-range tails cancel in the difference.
Out-of-range values give j outside 0..255 and are never counted; x == 1.0
(u == 256.0) belongs in bin 255 per torch.histc and is tallied by an exact
fp32 is_equal(256.0) pass, minus the one fp32 value above 1 (x = 1+2^-23)
whose x+1 rounds down to 2.0, tallied separately.
"""

import numpy as np

B, C, H, W = 64, 2, 512, 512
N_CORES = 8
P = 128
ELEMS_PER_CORE = (B // N_CORES) * C * H * W            # 4,194,304
FREE = ELEMS_PER_CORE // P                             # 32,768
F_CHUNK = 8192                                         # counting chunk (free dim)
N_CHUNKS = FREE // F_CHUNK                             # 4 per tensor
F_SUB = 2048                                           # prep sub-chunk
N_SUB = F_CHUNK // F_SUB                               # 4
TOT_CHUNKS = 2 * N_CHUNKS                              # pred chunks 0..3, gt 4..7
NB = 256
D_DVE = 90                                             # bins 0..89 on VectorE
N_ACT = NB - D_DVE + 1                                 # 167 sign passes: C_90..C_256
# acc_dve columns: per chunk: D_DVE bins, then per-sub u==256 and x==1+2^-23
DCOL_BINS = TOT_CHUNKS * D_DVE                         # 720
DCOL_EX = DCOL_BINS                                    # u==256 tallies start
DCOL_EX2 = DCOL_EX + TOT_CHUNKS * N_SUB                # x==1+2^-23 tallies start
NDCOL = DCOL_EX2 + TOT_CHUNKS * N_SUB                  # 784
NACOL = TOT_CHUNKS * N_ACT                             # 1336
EPS = 1e-8
X_ABOVE_ONE = float(np.float32(1.0) + np.float32(2.0 ** -23))

_CACHE = {}


def _build():
    import concourse.bacc as bacc
    import concourse.mybir as mybir
    import concourse.tile as tile

    f32 = mybir.dt.float32
    i16 = mybir.dt.int16
    i8 = mybir.dt.int8
    bf16 = mybir.dt.bfloat16
    op = mybir.AluOpType
    AF = mybir.ActivationFunctionType

    nc = bacc.Bacc("TRN2", target_bir_lowering=False, debug=False,
                   num_devices=N_CORES)
    pred_d = nc.dram_tensor("pred", [P, FREE], f32, kind="ExternalInput")
    gt_d = nc.dram_tensor("gt", [P, FREE], f32, kind="ExternalInput")
    ktab_d = nc.dram_tensor("ktab", [P, N_ACT], f32, kind="ExternalInput")
    od_d = nc.dram_tensor("od", [P, NDCOL], f32, kind="ExternalOutput")
    oa_d = nc.dram_tensor("oa", [P, NACOL], f32, kind="ExternalOutput")

    with tile.TileContext(nc) as tc:
        with (
            tc.tile_pool(name="xp", bufs=2) as xpool,
            tc.tile_pool(name="up", bufs=2) as upool,
            tc.tile_pool(name="jp", bufs=2) as jpool,
            tc.tile_pool(name="tp", bufs=2) as tpool,
            tc.tile_pool(name="tap", bufs=3) as tapool,
            tc.tile_pool(name="sp", bufs=2) as spool,
            tc.tile_pool(name="ap", bufs=1) as apool,
        ):
            ktab = apool.tile([P, N_ACT], f32)
            nc.sync.dma_start(ktab[:], ktab_d.ap())
            acc_d = apool.tile([P, NDCOL], f32)
            acc_a = apool.tile([P, NACOL], f32)
            for t_i, src in ((0, pred_d), (1, gt_d)):
                for q in range(N_CHUNKS):
                    c = t_i * N_CHUNKS + q
                    j2 = jpool.tile([P, F_CHUNK], i16, tag="j2")
                    for s in range(N_SUB):
                        lo = q * F_CHUNK + s * F_SUB
                        sl = slice(s * F_SUB, (s + 1) * F_SUB)
                        x_sub = xpool.tile([P, F_SUB], f32, tag="x")
                        nc.sync.dma_start(x_sub[:], src.ap()[:, lo:lo + F_SUB])
                        u_sub = upool.tile([P, F_SUB], f32, tag="u")
                        nc.vector.tensor_scalar(
                            u_sub[:], x_sub[:], 1.0, 128.0, op.add, op.mult)
                        r_sub = upool.tile([P, F_SUB], i16, tag="r")
                        nc.vector.tensor_copy(r_sub[:], u_sub[:])
                        m_sub = upool.tile([P, F_SUB], i16, tag="m")
                        nc.vector.tensor_tensor(m_sub[:], u_sub[:], r_sub[:], op.is_lt)
                        nc.vector.tensor_tensor(j2[:, sl], r_sub[:], m_sub[:], op.subtract)
                        # u == 256.0 detector (x in {0.99999994, 1.0, 1+2^-23})
                        t5 = spool.tile([P, F_SUB], bf16, tag="t5")
                        nc.vector.tensor_scalar(
                            t5[:], u_sub[:], 256.0, None, op.is_equal, op.add,
                            accum_out=acc_d[:, DCOL_EX + c * N_SUB + s:
                                            DCOL_EX + c * N_SUB + s + 1])
                        # x == 1+2^-23 (only x > 1 with u == 256.0; reference
                        # excludes it via in_range)
                        t6 = spool.tile([P, F_SUB], bf16, tag="t6")
                        nc.vector.tensor_scalar(
                            t6[:], x_sub[:], X_ABOVE_ONE, None, op.is_equal, op.add,
                            accum_out=acc_d[:, DCOL_EX2 + c * N_SUB + s:
                                            DCOL_EX2 + c * N_SUB + s + 1])
                    # VectorE: bins 0..D_DVE-1
                    for k in range(D_DVE):
                        trash = tpool.tile([P, F_CHUNK], i16, tag="trash")
                        nc.vector.tensor_scalar(
                            trash[:], j2[:], float(k), None,
                            op.is_equal, op.add,
                            accum_out=acc_d[:, c * D_DVE + k:c * D_DVE + k + 1])
                    # ScalarE: cumulative sign sums for C_{D_DVE}..C_256
                    for i in range(N_ACT):
                        trash_a = tapool.tile([P, F_CHUNK], i8, tag="trasha")
                        nc.scalar.activation(
                            trash_a[:], j2[:], AF.Sign,
                            bias=ktab[:, i:i + 1], scale=1.0,
                            accum_out=acc_a[:, c * N_ACT + i:c * N_ACT + i + 1])
            nc.sync.dma_start(od_d.ap(), acc_d[:])
            nc.sync.dma_start(oa_d.ap(), acc_a[:])
    nc.compile()
    return nc


def _get_nc():
    if "nc" not in _CACHE:
        _CACHE["nc"] = _build()
    return _CACHE["nc"]


def _ktab():
    # bias for boundary k: -(k - 0.5), k = D_DVE .. 256
    ks = np.arange(D_DVE, NB + 1, dtype=np.float64)
    vals = -(ks - 0.5)
    return np.tile(vals.astype(np.float32), (P, 1))


def _shard(arr):
    """[64, 2, 512, 512] f32 -> list of 8 per-core [128, 32768] arrays."""
    a = np.ascontiguousarray(np.asarray(arr, dtype=np.float32))
    per = B // N_CORES
    return [a[i * per:(i + 1) * per].reshape(P, FREE) for i in range(N_CORES)]


def _entropy_diff_from_hists(hp, hg):
    """Mirror reference._entropy in float32 on CPU via jax."""
    import jax
    import jax.numpy as jnp

    cpu = jax.devices("cpu")[0]
    with jax.default_device(cpu):
        def ent(h):
            h = jnp.asarray(np.asarray(h, dtype=np.float32))
            prob = h / jnp.sum(h) + np.float32(EPS)
            return -jnp.sum(prob * jnp.log(prob))
        out = jnp.abs(ent(hp) - ent(hg))
        return np.asarray(out).astype(np.float32).reshape(())


def kernel(predicted_ab, ground_truth_ab):
    from concourse import bass_utils

    nc = _get_nc()
    preds = _shard(predicted_ab)
    gts = _shard(ground_truth_ab)
    ktab = _ktab()
    in_maps = [{"pred": preds[i], "gt": gts[i], "ktab": ktab}
               for i in range(N_CORES)]
    res = bass_utils.run_bass_kernel_spmd(nc, in_maps, core_ids=list(range(N_CORES)))

    hist = np.zeros((2, NB), dtype=np.int64)
    extra = np.zeros(2, dtype=np.int64)
    extra2 = np.zeros(2, dtype=np.int64)
    # sign sums per tensor per boundary
    ssum = np.zeros((2, N_ACT), dtype=np.int64)
    for cidx in range(N_CORES):
        od = np.asarray(res.results[cidx]["od"], dtype=np.float64)
        oa = np.asarray(res.results[cidx]["oa"], dtype=np.float64)
        for t in range(2):
            for q in range(N_CHUNKS):
                c = t * N_CHUNKS + q
                hist[t, :D_DVE] += od[:, c * D_DVE:(c + 1) * D_DVE] \
                    .sum(axis=0).round().astype(np.int64)
                extra[t] += int(od[:, DCOL_EX + c * N_SUB:
                                   DCOL_EX + (c + 1) * N_SUB].sum().round())
                extra2[t] += int(od[:, DCOL_EX2 + c * N_SUB:
                                    DCOL_EX2 + (c + 1) * N_SUB].sum().round())
                ssum[t] += oa[:, c * N_ACT:(c + 1) * N_ACT] \
                    .sum(axis=0).round().astype(np.int64)
    total = np.int64(N_CORES) * ELEMS_PER_CORE
    # C_k = #{j >= k-0.5} = (total + sum(sign)) / 2, exact integers
    cum = (total + ssum) // 2
    assert np.all((total + ssum) % 2 == 0)
    hist[:, D_DVE:] = cum[:, :-1] - cum[:, 1:]
    # u == 256.0: x in {0.99999994, 1.0} are in-range -> bin 255; x == 1+2^-23
    # fails the reference's x <= 1 mask.
    hist[0, NB - 1] += extra[0] - extra2[0]
    hist[1, NB - 1] += extra[1] - extra2[1]
    return _entropy_diff_from_hists(hist[0], hist[1])


if __name__ == "__main__":
    rng = np.random.default_rng(0)
    p = rng.standard_normal((B, C, H, W)).astype(np.float32)
    g = rng.standard_normal((B, C, H, W)).astype(np.float32)
    got = kernel(p, g)

    def host_hist(x):
        x = x.ravel()
        u = (x.astype(np.float32) + np.float32(1.0)) * np.float32(128.0)
        idx = np.clip(np.floor(u.astype(np.float64)).astype(np.int64), 0, 255)
        m = (x >= -1.0) & (x <= 1.0)
        return np.bincount(idx[m], minlength=256)

    hp, hg = host_hist(p), host_hist(g)
    exp = _entropy_diff_from_hists(hp, hg)
    print("kernel:", got, "host:", exp, "absdiff:", abs(float(got) - float(exp)))

